# revision 6
# baseline (speedup 1.0000x reference)
"""Trainium2 Bass kernel for nn_DawsonIntegrate.

Computes, elementwise over x (f32):
    s = |x|;  near (s<=6): piecewise-Chebyshev table poly;  far (s>6): asymptotic
    plus, for x>0: (pi/2)*erfi(x) = sqrt(pi)*exp(x^2)*dawson(x).

Strategy (per core, data-parallel over 8 cores on the leading batch dim):
  ACT:   s=|x|, u=x^2, E=exp(u), L2=ln(u), w=exp(-L2)=1/x^2, mf=relu(s-6)
  DVE:   custom fused-Horner ops evaluate the 4 bin polynomials (exact
         monomial conversion of the Chebyshev table, delta-telescoped with
         in-op gating), the asymptotic far branch, and a 4-segment
         minimax fit of sqrt(pi)*dawson(x)/x (in u=x^2 or w=1/x^2).
  GPSIMD: tensor adds / final combine.
All table-dependent coefficients are computed on the host from cheb_G_neg
and baked into the program as immediates.
"""
import numpy as np

# ---------------------------------------------------------------- constants
DIV, DEG = 4, 8
CHEB_XMIN = -6.0
DELTA = 1.5
EULER_GAMMA = 0.5772156649015329
SQRT_PI = float(np.sqrt(np.pi))
DS1, DS2, DS3 = 1.5, 2.25, 4.0     # dawson segment boundaries (f32-exact)
XMAX = 9.7

FULL_SHAPE = (16, 2048, 1024)
N_CORES = 8
P = 128
TILE_N = 1024
ROWS_PER_CORE = FULL_SHAPE[0] // N_CORES  # 2
ELEMS_PER_CORE = ROWS_PER_CORE * FULL_SHAPE[1] * FULL_SHAPE[2]
N_TILES = ELEMS_PER_CORE // (P * TILE_N)  # 32


# ------------------------------------------------------- host-side numerics
def _dawsn_f64(x):
    """Dawson function via the stable all-positive erfi Maclaurin series,
    float64: D(x) = exp(-x^2) * sqrt(pi)/2 * erfi(x)."""
    x = np.asarray(x, dtype=np.float64)
    x2 = x * x
    t = x.copy()
    ssum = x.copy()
    for n in range(1, 400):
        t = t * x2 / n
        ssum = ssum + t / (2.0 * n + 1.0)
    # D = exp(-x^2) * series  (series = sqrt(pi)/2*erfi / ... cancels)
    return np.exp(-x2) * ssum


def _ref_bin_f32(s):
    v = (-s).astype(np.float32)
    t = ((v - np.float32(CHEB_XMIN)) / np.float32(DELTA)).astype(np.float32)
    b = np.ceil(t).astype(np.int32) - 1
    return np.clip(b, 0, DIV - 1)


def _effective_thresholds():
    ts = []
    for nominal in (1.5, 3.0, 4.5):
        lo = np.float32(nominal)
        cands = [lo]
        c = lo
        for _ in range(8):
            c = np.nextafter(c, np.float32(0), dtype=np.float32)
            cands.append(c)
        c = lo
        for _ in range(8):
            c = np.nextafter(c, np.float32(100), dtype=np.float32)
            cands.append(c)
        cands = np.sort(np.array(cands, dtype=np.float32))
        b = _ref_bin_f32(cands)
        assert b[0] == b[-1] + 1
        ts.append(float(cands[np.argmax(b == b[-1])]))
    return ts


def _minimax_fit(var, t, deg, iters=15):
    V = np.vander(var, deg + 1, increasing=True)
    w = 1.0 / np.abs(t)
    best = None
    for _ in range(iters):
        c, *_ = np.linalg.lstsq(V * w[:, None], t * w, rcond=None)
        r = np.abs((V @ c - t) / t)
        if best is None or r.max() < best[1]:
            best = (c, r.max())
        w = w * (1 + r / (r.max() + 1e-30))
    return best[0]


def _dawson_fit(xlo, xhi, basis, deg=6, n=6001):
    x = np.linspace(max(xlo, 1e-9), xhi, n)
    t = SQRT_PI * _dawsn_f64(x) / x
    var = x * x if basis == "u" else 1.0 / (x * x)
    return _minimax_fit(var, t, deg)


def build_constants(cheb_G_neg):
    t1, t2, t3 = _effective_thresholds()
    A = np.zeros((DIV, DEG))
    for b in range(DIV):
        mono_v = np.polynomial.chebyshev.cheb2poly(
            np.asarray(cheb_G_neg[b], dtype=np.float64))
        mono_v = np.concatenate([mono_v, np.zeros(DEG - len(mono_v))])
        A[b] = mono_v * ((-1.0) ** np.arange(DEG))
    return dict(
        t1=t1, t2=t2, t3=t3,
        base=A[3], d2=A[2] - A[3], d1=A[1] - A[2], d0=A[0] - A[1],
        r1=_dawson_fit(0, DS1, "u"),
        r2=_dawson_fit(DS1, DS2, "u"),
        r3=_dawson_fit(DS2, DS3, "w"),
        r4=_dawson_fit(DS3, XMAX, "w"),
        cf0=-0.25 * EULER_GAMMA - 0.5 * np.log(2.0),
        far_poly=(-5.0 / 32.0, 3.0 / 32.0, -1.0 / 8.0),  # cc, cb, ca (in w)
    )


# ------------------------------------------------- custom DVE op registration
_OPS_CACHE = {}


def _get_ops():
    if _OPS_CACHE:
        return _OPS_CACHE
    from concourse.dve_spec import (
        Spec, Src0, Src1, C0, C1, C2, C3, Zero, lower, select,
        _spill_c3_to_src1, _has_src1,
    )
    import concourse.dve_ops as dve_ops_mod
    from concourse.dve_ops import DveOp, OPS
    from concourse.dve_uop import DveOpSpec

    existing = {op.name for op in OPS}

    def reg(name, body, reference):
        spec = Spec(body=body, reference=reference)
        shas = {}
        for ver in ("v3", "v4"):
            shas[ver] = DveOpSpec(
                name=name, opcode=0, uops=lower(spec, ver=ver),
                rd1_en=_has_src1(spec),
            ).sha(ver)
        op = DveOp(name, spec, False, shas)
        if name not in existing:
            OPS.append(op)
        else:  # replace (idempotent re-import)
            for i, o in enumerate(OPS):
                if o.name == name:
                    OPS[i] = op
        # refresh the import-time snapshots keyed off OPS
        dve_ops_mod.CUSTOM_DVE_SPECS[name] = spec
        dve_ops_mod._SUB_OPCODE_FOR_NAME.clear()
        dve_ops_mod._SUB_OPCODE_FOR_NAME.update({
            o.name: dve_ops_mod._CUSTOM_DVE_ROW_BASE + i
            for i, o in enumerate(OPS)
        })
        assert max(dve_ops_mod._SUB_OPCODE_FOR_NAME.values()) < 0x20
        _OPS_CACHE[name] = op
        return op

    def _b(a):  # broadcast [P,1] -> [P,N]
        a = np.asarray(a)
        return a if a.ndim == 0 else a.reshape(a.shape[0], -1)[:, :1]

    # P4: h = ((s0*x + s1)*x + imm2)*x + in1   (4-coef Horner, single stream)
    reg("ANT_DI_P4",
        _spill_c3_to_src1(((Src0 * C0 + C1) * Src0 + C2) * Src0 + C3),
        lambda in0, in1, s0, s1, imm2:
            (((in0 * s0 + s1) * in0 + imm2) * in0 + _b(in1)).astype(np.float32))
    # P3: h = ((h*v + s0)*v + s1)*v + imm2    (3-coef Horner continue)
    reg("ANT_DI_P3",
        ((Src0 * Src1 + C0) * Src1 + C1) * Src1 + C2,
        lambda in0, in1, s0, s1, imm2:
            (((in0 * in1 + s0) * in1 + s1) * in1 + imm2).astype(np.float32))
    # PEND: out = h*v + s0
    reg("ANT_DI_PEND",
        Src0 * Src1 + C0,
        lambda in0, in1, s0, s1, imm2: (in0 * in1 + s0).astype(np.float32))
    # PGATE: out = (v >= s1) ? h*v + s0 : 0
    reg("ANT_DI_PGATE",
        select(Src1 >= C1, Src0 * Src1 + C0, Zero),
        lambda in0, in1, s0, s1, imm2:
            np.where(in1 >= s1, in0 * in1 + s0, 0.0).astype(np.float32))
    # DGATE0: out = (0 < v < s1) ? h*v : 0
    reg("ANT_DI_DGATE0",
        select((Src1 > Zero) & (Src1 < C1), Src0 * Src1, Zero),
        lambda in0, in1, s0, s1, imm2:
            np.where((in1 > 0) & (in1 < s1), in0 * in1, 0.0).astype(np.float32))
    # DGATE2: out = (s0 <= v < s1) ? h*v : 0
    reg("ANT_DI_DGATE2",
        select((Src1 >= C0) & (Src1 < C1), Src0 * Src1, Zero),
        lambda in0, in1, s0, s1, imm2:
            np.where((in1 >= s0) & (in1 < s1), in0 * in1, 0.0).astype(np.float32))
    # FAREND: out = h + L2*s0 + s1
    reg("ANT_DI_FAREND",
        Src0 + Src1 * C0 + C1,
        lambda in0, in1, s0, s1, imm2: (in0 + in1 * s0 + s1).astype(np.float32))
    return _OPS_CACHE


# ------------------------------------------------------------- kernel build
_BUILD_CACHE = {}


def _build_nc(C):
    import concourse.bacc as bacc
    import concourse.mybir as mybir
    from concourse.tile import TileContext

    ops = _get_ops()
    P4, P3 = ops["ANT_DI_P4"], ops["ANT_DI_P3"]
    PEND, PGATE = ops["ANT_DI_PEND"], ops["ANT_DI_PGATE"]
    DGATE0, DGATE2 = ops["ANT_DI_DGATE0"], ops["ANT_DI_DGATE2"]
    FAREND = ops["ANT_DI_FAREND"]

    f32 = mybir.dt.float32
    AF = mybir.ActivationFunctionType

    nc = bacc.Bacc("TRN2", target_bir_lowering=False)
    x_ext = nc.dram_tensor("x", [N_TILES, P, TILE_N], f32, kind="ExternalInput")
    y_ext = nc.dram_tensor("y", [N_TILES, P, TILE_N], f32, kind="ExternalOutput")

    def cst(v):
        return float(np.float32(v))

    def make_const(v):
        v = cst(v)
        key = (f32, v)
        if key not in nc.const_aps.aps:
            t = nc.alloc_sbuf_tensor(
                f"constr-{len(nc.const_aps.aps)}", [P, 1], f32)
            nc.gpsimd.memset(t.ap(), v)
            nc.const_aps.aps[key] = t.ap()
        return nc.const_aps.tensor(v, (P, 1), f32)

    # Pre-register every constant that must live in SBUF: the C3-spilled
    # 4th Horner coefficients and the activation bias.
    for v in (C["base"][4], C["d2"][4], C["d1"][4], C["d0"][4],
              C["r1"][3], C["r2"][3], C["r3"][3], C["r4"][3], -6.0):
        make_const(v)
    nc.all_engine_barrier()

    with TileContext(nc) as tc:
        with tc.tile_pool(name="io", bufs=3) as iop, \
             tc.tile_pool(name="acts", bufs=2) as actp, \
             tc.tile_pool(name="tmp", bufs=2) as tmpp:
            czero = make_const(0.0)
            for i in range(N_TILES):
                xt = iop.tile([P, TILE_N], f32, tag="x")
                nc.sync.dma_start(out=xt[:], in_=x_ext[i])

                s = actp.tile([P, TILE_N], f32, tag="s")
                u = actp.tile([P, TILE_N], f32, tag="u")
                E = actp.tile([P, TILE_N], f32, tag="E")
                L2 = actp.tile([P, TILE_N], f32, tag="L2")
                w = actp.tile([P, TILE_N], f32, tag="w")
                mf = actp.tile([P, TILE_N], f32, tag="mf")
                nc.scalar.activation(s[:], xt[:], AF.Abs)
                nc.scalar.activation(u[:], xt[:], AF.Square)
                nc.scalar.activation(E[:], u[:], AF.Exp)
                nc.scalar.activation(L2[:], u[:], AF.Ln)
                nc.scalar.activation(w[:], L2[:], AF.Exp, scale=-1.0)
                nc.scalar.activation(mf[:], s[:], AF.Relu, bias=-6.0)

                def c3ap(v):
                    return make_const(v)

                def poly8(var, coefs, tag):
                    h1 = tmpp.tile([P, TILE_N], f32, tag="h4")
                    nc.vector._custom_dve(
                        P4, out=h1[:], in0=var[:], in1=c3ap(coefs[4]),
                        s0=cst(coefs[7]), s1=cst(coefs[6]), imm2=cst(coefs[5]))
                    h2 = tmpp.tile([P, TILE_N], f32, tag="h3")
                    nc.vector._custom_dve(
                        P3, out=h2[:], in0=h1[:], in1=var[:],
                        s0=cst(coefs[3]), s1=cst(coefs[2]), imm2=cst(coefs[1]))
                    return h2

                def poly7(var, coefs, tag):
                    # deg-6: coefs[k] = coef of var^k, k=0..6
                    h1 = tmpp.tile([P, TILE_N], f32, tag="h4")
                    nc.vector._custom_dve(
                        P4, out=h1[:], in0=var[:], in1=c3ap(coefs[3]),
                        s0=cst(coefs[6]), s1=cst(coefs[5]), imm2=cst(coefs[4]))
                    h2 = tmpp.tile([P, TILE_N], f32, tag="h3")
                    nc.vector._custom_dve(
                        P3, out=h2[:], in0=h1[:], in1=var[:],
                        s0=cst(coefs[2]), s1=cst(coefs[1]), imm2=cst(coefs[0]))
                    return h2

                # ---- Q: base bin + gated deltas ----
                Q = tmpp.tile([P, TILE_N], f32, tag="Q")
                h = poly8(s, C["base"], "qb")
                nc.vector._custom_dve(PEND, out=Q[:], in0=h[:], in1=s[:],
                                      s0=cst(C["base"][0]))
                for dd, th, tg in ((C["d2"], C["t1"], "q2"),
                                   (C["d1"], C["t2"], "q1"),
                                   (C["d0"], C["t3"], "q0")):
                    h = poly8(s, dd, tg)
                    F = tmpp.tile([P, TILE_N], f32, tag="F")
                    nc.vector._custom_dve(PGATE, out=F[:], in0=h[:], in1=s[:],
                                          s0=cst(dd[0]), s1=cst(th))
                    nc.gpsimd.tensor_add(out=Q[:], in0=Q[:], in1=F[:])

                # ---- far branch ----
                cc, cb, ca = C["far_poly"]
                hf = tmpp.tile([P, TILE_N], f32, tag="hf")
                nc.vector._custom_dve(P4, out=hf[:], in0=w[:], in1=czero,
                                      s0=cst(cc), s1=cst(cb), imm2=cst(ca))
                yfar = tmpp.tile([P, TILE_N], f32, tag="yfar")
                nc.vector._custom_dve(FAREND, out=yfar[:], in0=hf[:], in1=L2[:],
                                      s0=-0.25, s1=cst(C["cf0"]))
                nc.vector.copy_predicated(
                    Q[:], mf[:].bitcast(mybir.dt.int32), yfar[:])

                # ---- Dawson: 4 gated segments ----
                D = tmpp.tile([P, TILE_N], f32, tag="D")
                h = poly7(u, C["r1"], "d1")
                nc.vector._custom_dve(DGATE0, out=D[:], in0=h[:], in1=xt[:],
                                      s1=cst(DS1))
                Dx = tmpp.tile([P, TILE_N], f32, tag="Dx")
                h = poly7(u, C["r2"], "d2")
                nc.vector._custom_dve(DGATE2, out=Dx[:], in0=h[:], in1=xt[:],
                                      s0=cst(DS1), s1=cst(DS2))
                nc.gpsimd.tensor_add(out=D[:], in0=D[:], in1=Dx[:])
                Dy = tmpp.tile([P, TILE_N], f32, tag="Dy")
                h = poly7(w, C["r3"], "d3")
                nc.vector._custom_dve(DGATE2, out=Dy[:], in0=h[:], in1=xt[:],
                                      s0=cst(DS2), s1=cst(DS3))
                Dz = tmpp.tile([P, TILE_N], f32, tag="Dz")
                h = poly7(w, C["r4"], "d4")
                nc.vector._custom_dve(PGATE, out=Dz[:], in0=h[:], in1=xt[:],
                                      s0=0.0, s1=cst(DS3))
                nc.gpsimd.tensor_add(out=Dy[:], in0=Dy[:], in1=Dz[:])
                nc.gpsimd.tensor_add(out=D[:], in0=D[:], in1=Dy[:])

                # ---- combine: out = Q + E*D ----
                z = tmpp.tile([P, TILE_N], f32, tag="z")
                nc.gpsimd.tensor_mul(out=z[:], in0=E[:], in1=D[:])
                outt = iop.tile([P, TILE_N], f32, tag="y")
                nc.gpsimd.tensor_add(out=outt[:], in0=Q[:], in1=z[:])
                nc.sync.dma_start(out=y_ext[i], in_=outt[:])
    nc.compile()
    return nc


def _get_nc(cheb_G_neg):
    key = np.asarray(cheb_G_neg, dtype=np.float32).tobytes()
    if key not in _BUILD_CACHE:
        C = build_constants(np.asarray(cheb_G_neg, dtype=np.float32))
        _BUILD_CACHE[key] = _build_nc(C)
    return _BUILD_CACHE[key]


# ------------------------------------------------------------- entry points
def _run(x, cheb_G_neg, **spmd_kwargs):
    from concourse.bass_utils import run_bass_kernel_spmd
    nc = _get_nc(cheb_G_neg)
    x = np.ascontiguousarray(np.asarray(x, dtype=np.float32))
    shards = x.reshape(N_CORES, N_TILES, P, TILE_N)
    in_maps = [{"x": shards[i]} for i in range(N_CORES)]
    res = run_bass_kernel_spmd(nc, in_maps, list(range(N_CORES)), **spmd_kwargs)
    out = np.stack([np.asarray(res.results[i]["y"]) for i in range(N_CORES)])
    return out.reshape(FULL_SHAPE), res


def kernel(x, cheb_G_neg):
    out, _ = _run(x, cheb_G_neg)
    return out


def kernel_timed(x, cheb_G_neg, **kw):
    return _run(x, cheb_G_neg, trace=True, **kw)


# revision 13
# speedup vs baseline: 1.1536x; 1.1536x over previous
"""Trainium2 Bass kernel for nn_DawsonIntegrate.

Computes, elementwise over x (f32):
    s = |x|;  near (s<=6): piecewise-Chebyshev table poly;  far (s>6): asymptotic
    plus, for x>0: (pi/2)*erfi(x) = sqrt(pi)*exp(x^2)*dawson(x).

Strategy (per core, data-parallel over 8 cores on the leading batch dim):
  ACT:   s=|x|, u=x^2, E=exp(u), L2=ln(u), w=exp(-L2)=1/x^2, mf=relu(s-6)
  DVE:   custom fused-Horner ops evaluate the 4 bin polynomials (exact
         monomial conversion of the Chebyshev table, delta-telescoped with
         in-op gating), the asymptotic far branch, and a 4-segment
         minimax fit of sqrt(pi)*dawson(x)/x (in u=x^2 or w=1/x^2).
  GPSIMD: tensor adds / final combine.
All table-dependent coefficients are computed on the host from cheb_G_neg
and baked into the program as immediates.
"""
import numpy as np

# ---------------------------------------------------------------- constants
DIV, DEG = 4, 8
CHEB_XMIN = -6.0
DELTA = 1.5
EULER_GAMMA = 0.5772156649015329
SQRT_PI = float(np.sqrt(np.pi))
DS1, DS2 = 1.625, 2.75             # dawson segment boundaries (f32-exact)
XMAX = 9.7

FULL_SHAPE = (16, 2048, 1024)
N_CORES = 8
P = 128
TILE_N = 1024
ROWS_PER_CORE = FULL_SHAPE[0] // N_CORES  # 2
ELEMS_PER_CORE = ROWS_PER_CORE * FULL_SHAPE[1] * FULL_SHAPE[2]
N_TILES = ELEMS_PER_CORE // (P * TILE_N)  # 32


# ------------------------------------------------------- host-side numerics
def _dawsn_f64(x):
    """Dawson function via the stable all-positive erfi Maclaurin series,
    float64: D(x) = exp(-x^2) * sqrt(pi)/2 * erfi(x)."""
    x = np.asarray(x, dtype=np.float64)
    x2 = x * x
    t = x.copy()
    ssum = x.copy()
    for n in range(1, 400):
        t = t * x2 / n
        ssum = ssum + t / (2.0 * n + 1.0)
    # D = exp(-x^2) * series  (series = sqrt(pi)/2*erfi / ... cancels)
    return np.exp(-x2) * ssum


def _ref_bin_f32(s):
    v = (-s).astype(np.float32)
    t = ((v - np.float32(CHEB_XMIN)) / np.float32(DELTA)).astype(np.float32)
    b = np.ceil(t).astype(np.int32) - 1
    return np.clip(b, 0, DIV - 1)


def _effective_thresholds():
    ts = []
    for nominal in (1.5, 3.0, 4.5):
        lo = np.float32(nominal)
        cands = [lo]
        c = lo
        for _ in range(8):
            c = np.nextafter(c, np.float32(0), dtype=np.float32)
            cands.append(c)
        c = lo
        for _ in range(8):
            c = np.nextafter(c, np.float32(100), dtype=np.float32)
            cands.append(c)
        cands = np.sort(np.array(cands, dtype=np.float32))
        b = _ref_bin_f32(cands)
        assert b[0] == b[-1] + 1
        ts.append(float(cands[np.argmax(b == b[-1])]))
    return ts


def _minimax_fit(var, t, deg, iters=15):
    V = np.vander(var, deg + 1, increasing=True)
    w = 1.0 / np.abs(t)
    best = None
    for _ in range(iters):
        c, *_ = np.linalg.lstsq(V * w[:, None], t * w, rcond=None)
        r = np.abs((V @ c - t) / t)
        if best is None or r.max() < best[1]:
            best = (c, r.max())
        w = w * (1 + r / (r.max() + 1e-30))
    return best[0]


def _dawson_fit(xlo, xhi, basis, deg=6, n=6001):
    x = np.linspace(max(xlo, 1e-9), xhi, n)
    t = SQRT_PI * _dawsn_f64(x) / x
    var = x * x if basis == "u" else 1.0 / (x * x)
    return _minimax_fit(var, t, deg)


def build_constants(cheb_G_neg):
    t1, t2, t3 = _effective_thresholds()
    A = np.zeros((DIV, DEG))
    for b in range(DIV):
        mono_v = np.polynomial.chebyshev.cheb2poly(
            np.asarray(cheb_G_neg[b], dtype=np.float64))
        mono_v = np.concatenate([mono_v, np.zeros(DEG - len(mono_v))])
        A[b] = mono_v * ((-1.0) ** np.arange(DEG))
    return dict(
        t1=t1, t2=t2, t3=t3,
        base=A[3], d2=A[2] - A[3], d1=A[1] - A[2], d0=A[0] - A[1],
        r1=_dawson_fit(0, DS1, "u"),
        r2=_dawson_fit(DS1, DS2, "w"),
        r3=_dawson_fit(DS2, XMAX, "w"),
        cf0=-0.25 * EULER_GAMMA - 0.5 * np.log(2.0),
        far_poly=(-5.0 / 32.0, 3.0 / 32.0, -1.0 / 8.0),  # cc, cb, ca (in w)
    )


# ------------------------------------------------- custom DVE op registration
_OPS_CACHE = {}


def _get_ops():
    if _OPS_CACHE:
        return _OPS_CACHE
    from concourse.dve_spec import (
        Spec, Src0, Src1, C0, C1, C2, C3, Zero, lower, select,
        _spill_c3_to_src1, _has_src1,
    )
    import concourse.dve_ops as dve_ops_mod
    from concourse.dve_ops import DveOp, OPS
    from concourse.dve_uop import DveOpSpec

    existing = {op.name for op in OPS}

    def reg(name, body, reference):
        spec = Spec(body=body, reference=reference)
        shas = {}
        for ver in ("v3", "v4"):
            shas[ver] = DveOpSpec(
                name=name, opcode=0, uops=lower(spec, ver=ver),
                rd1_en=_has_src1(spec),
            ).sha(ver)
        op = DveOp(name, spec, False, shas)
        if name not in existing:
            OPS.append(op)
        else:  # replace (idempotent re-import)
            for i, o in enumerate(OPS):
                if o.name == name:
                    OPS[i] = op
        # refresh the import-time snapshots keyed off OPS
        dve_ops_mod.CUSTOM_DVE_SPECS[name] = spec
        dve_ops_mod._SUB_OPCODE_FOR_NAME.clear()
        dve_ops_mod._SUB_OPCODE_FOR_NAME.update({
            o.name: dve_ops_mod._CUSTOM_DVE_ROW_BASE + i
            for i, o in enumerate(OPS)
        })
        assert max(dve_ops_mod._SUB_OPCODE_FOR_NAME.values()) < 0x20
        _OPS_CACHE[name] = op
        return op

    def _b(a):  # broadcast [P,1] -> [P,N]
        a = np.asarray(a)
        return a if a.ndim == 0 else a.reshape(a.shape[0], -1)[:, :1]

    # P4: h = ((s0*x + s1)*x + imm2)*x + in1   (4-coef Horner, single stream)
    reg("ANT_DI_P4",
        _spill_c3_to_src1(((Src0 * C0 + C1) * Src0 + C2) * Src0 + C3),
        lambda in0, in1, s0, s1, imm2:
            (((in0 * s0 + s1) * in0 + imm2) * in0 + _b(in1)).astype(np.float32))
    # P3: h = ((h*v + s0)*v + s1)*v + imm2    (3-coef Horner continue)
    reg("ANT_DI_P3",
        ((Src0 * Src1 + C0) * Src1 + C1) * Src1 + C2,
        lambda in0, in1, s0, s1, imm2:
            (((in0 * in1 + s0) * in1 + s1) * in1 + imm2).astype(np.float32))
    # PEND: out = h*v + s0
    reg("ANT_DI_PEND",
        Src0 * Src1 + C0,
        lambda in0, in1, s0, s1, imm2: (in0 * in1 + s0).astype(np.float32))
    # PGATE: out = (v >= s1) ? h*v + s0 : 0
    reg("ANT_DI_PGATE",
        select(Src1 >= C1, Src0 * Src1 + C0, Zero),
        lambda in0, in1, s0, s1, imm2:
            np.where(in1 >= s1, in0 * in1 + s0, 0.0).astype(np.float32))
    # DGATE0: out = (0 < v < s1) ? h*v : 0
    reg("ANT_DI_DGATE0",
        select((Src1 > Zero) & (Src1 < C1), Src0 * Src1, Zero),
        lambda in0, in1, s0, s1, imm2:
            np.where((in1 > 0) & (in1 < s1), in0 * in1, 0.0).astype(np.float32))
    # DGATE2: out = (s0 <= v < s1) ? h*v : 0
    reg("ANT_DI_DGATE2",
        select((Src1 >= C0) & (Src1 < C1), Src0 * Src1, Zero),
        lambda in0, in1, s0, s1, imm2:
            np.where((in1 >= s0) & (in1 < s1), in0 * in1, 0.0).astype(np.float32))
    # P4T: out = ((s0*x + s1)*x + imm2)*x + in1[tensor]  (Horner + tensor tail)
    reg("ANT_DI_P4T",
        ((Src0 * C0 + C1) * Src0 + C2) * Src0 + Src1,
        lambda in0, in1, s0, s1, imm2:
            (((in0 * s0 + s1) * in0 + imm2) * in0 + in1).astype(np.float32))
    return _OPS_CACHE


# ------------------------------------------------------------- kernel build
_BUILD_CACHE = {}


def _build_nc(C):
    import concourse.bacc as bacc
    import concourse.mybir as mybir
    from concourse.tile import TileContext

    ops = _get_ops()
    P4, P3 = ops["ANT_DI_P4"], ops["ANT_DI_P3"]
    PEND, PGATE = ops["ANT_DI_PEND"], ops["ANT_DI_PGATE"]
    DGATE0, DGATE2 = ops["ANT_DI_DGATE0"], ops["ANT_DI_DGATE2"]
    P4T = ops["ANT_DI_P4T"]

    f32 = mybir.dt.float32
    AF = mybir.ActivationFunctionType

    nc = bacc.Bacc("TRN2", target_bir_lowering=False)
    x_ext = nc.dram_tensor("x", [N_TILES, P, TILE_N], f32, kind="ExternalInput")
    y_ext = nc.dram_tensor("y", [N_TILES, P, TILE_N], f32, kind="ExternalOutput")

    def cst(v):
        return float(np.float32(v))

    def make_const(v):
        v = cst(v)
        key = (f32, v)
        if key not in nc.const_aps.aps:
            t = nc.alloc_sbuf_tensor(
                f"constr-{len(nc.const_aps.aps)}", [P, 1], f32)
            nc.gpsimd.memset(t.ap(), v)
            nc.const_aps.aps[key] = t.ap()
        return nc.const_aps.tensor(v, (P, 1), f32)

    # Pre-register every constant that must live in SBUF: the C3-spilled
    # 4th Horner coefficients and the activation bias.
    for v in (C["base"][4], C["d2"][4], C["d1"][4], C["d0"][4],
              C["r1"][3], C["r2"][3], C["r3"][3], -6.0, C["cf0"]):
        make_const(v)
    nc.all_engine_barrier()

    with TileContext(nc) as tc:
        with tc.tile_pool(name="io", bufs=3) as iop, \
             tc.tile_pool(name="acts", bufs=2) as actp, \
             tc.tile_pool(name="tmp", bufs=2) as tmpp:
            czero = make_const(0.0)
            for i in range(N_TILES):
                xt = iop.tile([P, TILE_N], f32, tag="x")
                nc.sync.dma_start(out=xt[:], in_=x_ext[i])

                s = actp.tile([P, TILE_N], f32, tag="s")
                u = actp.tile([P, TILE_N], f32, tag="u")
                E = actp.tile([P, TILE_N], f32, tag="E")
                L2 = actp.tile([P, TILE_N], f32, tag="L2")
                w = actp.tile([P, TILE_N], f32, tag="w")
                mf = actp.tile([P, TILE_N], f32, tag="mf")
                yfl = actp.tile([P, TILE_N], f32, tag="yfl")
                nc.scalar.activation(s[:], xt[:], AF.Abs)
                nc.scalar.activation(u[:], xt[:], AF.Square)
                nc.scalar.activation(E[:], u[:], AF.Exp)
                nc.scalar.activation(L2[:], u[:], AF.Ln)
                nc.scalar.activation(w[:], L2[:], AF.Exp, scale=-1.0)
                nc.scalar.activation(mf[:], s[:], AF.Relu, bias=-6.0)
                # yfl = -0.25*L2 + cf0 (the linear part of the far branch)
                nc.scalar.activation(yfl[:], L2[:], AF.Identity,
                                     scale=-0.25, bias=cst(C["cf0"]))

                def c3ap(v):
                    return make_const(v)

                def poly8(var, coefs, tag):
                    h1 = tmpp.tile([P, TILE_N], f32, tag="h4")
                    nc.vector._custom_dve(
                        P4, out=h1[:], in0=var[:], in1=c3ap(coefs[4]),
                        s0=cst(coefs[7]), s1=cst(coefs[6]), imm2=cst(coefs[5]))
                    h2 = tmpp.tile([P, TILE_N], f32, tag="h3")
                    nc.vector._custom_dve(
                        P3, out=h2[:], in0=h1[:], in1=var[:],
                        s0=cst(coefs[3]), s1=cst(coefs[2]), imm2=cst(coefs[1]))
                    return h2

                def poly7(var, coefs, tag):
                    # deg-6: coefs[k] = coef of var^k, k=0..6
                    h1 = tmpp.tile([P, TILE_N], f32, tag="h4")
                    nc.vector._custom_dve(
                        P4, out=h1[:], in0=var[:], in1=c3ap(coefs[3]),
                        s0=cst(coefs[6]), s1=cst(coefs[5]), imm2=cst(coefs[4]))
                    h2 = tmpp.tile([P, TILE_N], f32, tag="h3")
                    nc.vector._custom_dve(
                        P3, out=h2[:], in0=h1[:], in1=var[:],
                        s0=cst(coefs[2]), s1=cst(coefs[1]), imm2=cst(coefs[0]))
                    return h2

                # ---- Q: base bin + gated deltas ----
                Q = tmpp.tile([P, TILE_N], f32, tag="Q")
                h = poly8(s, C["base"], "qb")
                nc.vector._custom_dve(PEND, out=Q[:], in0=h[:], in1=s[:],
                                      s0=cst(C["base"][0]))
                for dd, th, tg in ((C["d2"], C["t1"], "q2"),
                                   (C["d1"], C["t2"], "q1"),
                                   (C["d0"], C["t3"], "q0")):
                    h = poly8(s, dd, tg)
                    F = tmpp.tile([P, TILE_N], f32, tag="F")
                    nc.vector._custom_dve(PGATE, out=F[:], in0=h[:], in1=s[:],
                                          s0=cst(dd[0]), s1=cst(th))
                    nc.gpsimd.tensor_add(out=Q[:], in0=Q[:], in1=F[:])

                # ---- far branch: yfar = far_poly(w)*w + yfl, merged by mf ----
                cc, cb, ca = C["far_poly"]
                yfar = tmpp.tile([P, TILE_N], f32, tag="yfar")
                nc.vector._custom_dve(P4T, out=yfar[:], in0=w[:], in1=yfl[:],
                                      s0=cst(cc), s1=cst(cb), imm2=cst(ca))
                nc.vector.copy_predicated(
                    Q[:], mf[:].bitcast(mybir.dt.int32), yfar[:])

                # ---- Dawson: 3 gated segments ----
                D = tmpp.tile([P, TILE_N], f32, tag="D")
                h = poly7(u, C["r1"], "d1")
                nc.vector._custom_dve(DGATE0, out=D[:], in0=h[:], in1=xt[:],
                                      s1=cst(DS1))
                Dx = tmpp.tile([P, TILE_N], f32, tag="Dx")
                h = poly7(w, C["r2"], "d2")
                nc.vector._custom_dve(DGATE2, out=Dx[:], in0=h[:], in1=xt[:],
                                      s0=cst(DS1), s1=cst(DS2))
                nc.gpsimd.tensor_add(out=D[:], in0=D[:], in1=Dx[:])
                Dy = tmpp.tile([P, TILE_N], f32, tag="Dy")
                h = poly7(w, C["r3"], "d3")
                nc.vector._custom_dve(PGATE, out=Dy[:], in0=h[:], in1=xt[:],
                                      s0=0.0, s1=cst(DS2))
                nc.gpsimd.tensor_add(out=D[:], in0=D[:], in1=Dy[:])

                # ---- combine: out = Q + E*D ----
                z = tmpp.tile([P, TILE_N], f32, tag="z")
                nc.gpsimd.tensor_mul(out=z[:], in0=E[:], in1=D[:])
                outt = iop.tile([P, TILE_N], f32, tag="y")
                nc.gpsimd.tensor_add(out=outt[:], in0=Q[:], in1=z[:])
                nc.sync.dma_start(out=y_ext[i], in_=outt[:])
    nc.compile()
    return nc


def _get_nc(cheb_G_neg):
    key = np.asarray(cheb_G_neg, dtype=np.float32).tobytes()
    if key not in _BUILD_CACHE:
        C = build_constants(np.asarray(cheb_G_neg, dtype=np.float32))
        _BUILD_CACHE[key] = _build_nc(C)
    return _BUILD_CACHE[key]


# ------------------------------------------------------------- entry points
def _run(x, cheb_G_neg, **spmd_kwargs):
    from concourse.bass_utils import run_bass_kernel_spmd
    nc = _get_nc(cheb_G_neg)
    x = np.ascontiguousarray(np.asarray(x, dtype=np.float32))
    shards = x.reshape(N_CORES, N_TILES, P, TILE_N)
    in_maps = [{"x": shards[i]} for i in range(N_CORES)]
    res = run_bass_kernel_spmd(nc, in_maps, list(range(N_CORES)), **spmd_kwargs)
    out = np.stack([np.asarray(res.results[i]["y"]) for i in range(N_CORES)])
    return out.reshape(FULL_SHAPE), res


def kernel(x, cheb_G_neg):
    out, _ = _run(x, cheb_G_neg)
    return out


def kernel_timed(x, cheb_G_neg, **kw):
    return _run(x, cheb_G_neg, trace=True, **kw)


# revision 14
# speedup vs baseline: 1.3806x; 1.1967x over previous
"""Trainium2 Bass kernel for nn_DawsonIntegrate.

Computes, elementwise over x (f32):
    s = |x|;  near (s<=6): piecewise-Chebyshev table poly;  far (s>6): asymptotic
    plus, for x>0: (pi/2)*erfi(x) = sqrt(pi)*exp(x^2)*dawson(x).

Strategy (per core, data-parallel over 8 cores on the leading batch dim):
  ACT:   s=|x|, u=x^2, E=exp(u), L2=ln(u), w=exp(-L2)=1/x^2, mf=relu(s-6)
  DVE:   custom fused-Horner ops evaluate the 4 bin polynomials (exact
         monomial conversion of the Chebyshev table, delta-telescoped with
         in-op gating), the asymptotic far branch, and a 4-segment
         minimax fit of sqrt(pi)*dawson(x)/x (in u=x^2 or w=1/x^2).
  GPSIMD: tensor adds / final combine.
All table-dependent coefficients are computed on the host from cheb_G_neg
and baked into the program as immediates.
"""
import numpy as np

# ---------------------------------------------------------------- constants
DIV, DEG = 4, 8
CHEB_XMIN = -6.0
DELTA = 1.5
EULER_GAMMA = 0.5772156649015329
SQRT_PI = float(np.sqrt(np.pi))
DS1 = 2.25                         # dawson segment boundary (f32-exact)
XMAX = 9.7

FULL_SHAPE = (16, 2048, 1024)
N_CORES = 8
P = 128
TILE_N = 1024
ROWS_PER_CORE = FULL_SHAPE[0] // N_CORES  # 2
ELEMS_PER_CORE = ROWS_PER_CORE * FULL_SHAPE[1] * FULL_SHAPE[2]
N_TILES = ELEMS_PER_CORE // (P * TILE_N)  # 32


# ------------------------------------------------------- host-side numerics
def _dawsn_f64(x):
    """Dawson function via the stable all-positive erfi Maclaurin series,
    float64: D(x) = exp(-x^2) * sqrt(pi)/2 * erfi(x)."""
    x = np.asarray(x, dtype=np.float64)
    x2 = x * x
    t = x.copy()
    ssum = x.copy()
    for n in range(1, 400):
        t = t * x2 / n
        ssum = ssum + t / (2.0 * n + 1.0)
    # D = exp(-x^2) * series  (series = sqrt(pi)/2*erfi / ... cancels)
    return np.exp(-x2) * ssum


def _ref_bin_f32(s):
    v = (-s).astype(np.float32)
    t = ((v - np.float32(CHEB_XMIN)) / np.float32(DELTA)).astype(np.float32)
    b = np.ceil(t).astype(np.int32) - 1
    return np.clip(b, 0, DIV - 1)


def _effective_thresholds():
    ts = []
    for nominal in (1.5, 3.0, 4.5):
        lo = np.float32(nominal)
        cands = [lo]
        c = lo
        for _ in range(8):
            c = np.nextafter(c, np.float32(0), dtype=np.float32)
            cands.append(c)
        c = lo
        for _ in range(8):
            c = np.nextafter(c, np.float32(100), dtype=np.float32)
            cands.append(c)
        cands = np.sort(np.array(cands, dtype=np.float32))
        b = _ref_bin_f32(cands)
        assert b[0] == b[-1] + 1
        ts.append(float(cands[np.argmax(b == b[-1])]))
    return ts


def _minimax_fit(var, t, deg, iters=15):
    V = np.vander(var, deg + 1, increasing=True)
    w = 1.0 / np.abs(t)
    best = None
    for _ in range(iters):
        c, *_ = np.linalg.lstsq(V * w[:, None], t * w, rcond=None)
        r = np.abs((V @ c - t) / t)
        if best is None or r.max() < best[1]:
            best = (c, r.max())
        w = w * (1 + r / (r.max() + 1e-30))
    return best[0]


def _dawson_fit(xlo, xhi, basis, deg=6, n=6001):
    x = np.linspace(max(xlo, 1e-9), xhi, n)
    t = SQRT_PI * _dawsn_f64(x) / x
    var = x * x if basis == "u" else 1.0 / (x * x)
    return _minimax_fit(var, t, deg)


def build_constants(cheb_G_neg):
    t1, t2, t3 = _effective_thresholds()
    A = np.zeros((DIV, DEG))
    for b in range(DIV):
        mono_v = np.polynomial.chebyshev.cheb2poly(
            np.asarray(cheb_G_neg[b], dtype=np.float64))
        mono_v = np.concatenate([mono_v, np.zeros(DEG - len(mono_v))])
        A[b] = mono_v * ((-1.0) ** np.arange(DEG))
    return dict(
        t1=t1, t2=t2, t3=t3,
        base=A[3], d2=A[2] - A[3], d1=A[1] - A[2], d0=A[0] - A[1],
        r1=_dawson_fit(0, DS1, "u", deg=8),
        r2=_dawson_fit(DS1, XMAX, "w", deg=8),
        cf0=-0.25 * EULER_GAMMA - 0.5 * np.log(2.0),
        far_poly=(-5.0 / 32.0, 3.0 / 32.0, -1.0 / 8.0),  # cc, cb, ca (in w)
    )


# ------------------------------------------------- custom DVE op registration
_OPS_CACHE = {}


def _get_ops():
    if _OPS_CACHE:
        return _OPS_CACHE
    from concourse.dve_spec import (
        Spec, Src0, Src1, C0, C1, C2, C3, Zero, lower, select,
        _spill_c3_to_src1, _has_src1,
    )
    import concourse.dve_ops as dve_ops_mod
    from concourse.dve_ops import DveOp, OPS
    from concourse.dve_uop import DveOpSpec

    existing = {op.name for op in OPS}

    def reg(name, body, reference):
        spec = Spec(body=body, reference=reference)
        shas = {}
        for ver in ("v3", "v4"):
            shas[ver] = DveOpSpec(
                name=name, opcode=0, uops=lower(spec, ver=ver),
                rd1_en=_has_src1(spec),
            ).sha(ver)
        op = DveOp(name, spec, False, shas)
        if name not in existing:
            OPS.append(op)
        else:  # replace (idempotent re-import)
            for i, o in enumerate(OPS):
                if o.name == name:
                    OPS[i] = op
        # refresh the import-time snapshots keyed off OPS
        dve_ops_mod.CUSTOM_DVE_SPECS[name] = spec
        dve_ops_mod._SUB_OPCODE_FOR_NAME.clear()
        dve_ops_mod._SUB_OPCODE_FOR_NAME.update({
            o.name: dve_ops_mod._CUSTOM_DVE_ROW_BASE + i
            for i, o in enumerate(OPS)
        })
        assert max(dve_ops_mod._SUB_OPCODE_FOR_NAME.values()) < 0x20
        _OPS_CACHE[name] = op
        return op

    def _b(a):  # broadcast [P,1] -> [P,N]
        a = np.asarray(a)
        return a if a.ndim == 0 else a.reshape(a.shape[0], -1)[:, :1]

    # P4: h = ((s0*x + s1)*x + imm2)*x + in1   (4-coef Horner, single stream)
    reg("ANT_DI_P4",
        _spill_c3_to_src1(((Src0 * C0 + C1) * Src0 + C2) * Src0 + C3),
        lambda in0, in1, s0, s1, imm2:
            (((in0 * s0 + s1) * in0 + imm2) * in0 + _b(in1)).astype(np.float32))
    # P3: h = ((h*v + s0)*v + s1)*v + imm2    (3-coef Horner continue)
    reg("ANT_DI_P3",
        ((Src0 * Src1 + C0) * Src1 + C1) * Src1 + C2,
        lambda in0, in1, s0, s1, imm2:
            (((in0 * in1 + s0) * in1 + s1) * in1 + imm2).astype(np.float32))
    # PEND: out = h*v + s0
    reg("ANT_DI_PEND",
        Src0 * Src1 + C0,
        lambda in0, in1, s0, s1, imm2: (in0 * in1 + s0).astype(np.float32))
    # PGATE: out = (v >= s1) ? h*v + s0 : 0
    reg("ANT_DI_PGATE",
        select(Src1 >= C1, Src0 * Src1 + C0, Zero),
        lambda in0, in1, s0, s1, imm2:
            np.where(in1 >= s1, in0 * in1 + s0, 0.0).astype(np.float32))
    # DGATE0: out = (0 < v < s1) ? h*v : 0
    reg("ANT_DI_DGATE0",
        select((Src1 > Zero) & (Src1 < C1), Src0 * Src1, Zero),
        lambda in0, in1, s0, s1, imm2:
            np.where((in1 > 0) & (in1 < s1), in0 * in1, 0.0).astype(np.float32))
    # DGATE2: out = (s0 <= v < s1) ? h*v : 0
    reg("ANT_DI_DGATE2",
        select((Src1 >= C0) & (Src1 < C1), Src0 * Src1, Zero),
        lambda in0, in1, s0, s1, imm2:
            np.where((in1 >= s0) & (in1 < s1), in0 * in1, 0.0).astype(np.float32))
    # P3F: h = (s0*x + s1)*x + imm2   (fresh 3-coef Horner, single stream)
    reg("ANT_DI_P3F",
        (Src0 * C0 + C1) * Src0 + C2,
        lambda in0, in1, s0, s1, imm2:
            ((in0 * s0 + s1) * in0 + imm2).astype(np.float32))
    # P2E: out = (h*v + s0)*v + s1
    reg("ANT_DI_P2E",
        (Src0 * Src1 + C0) * Src1 + C1,
        lambda in0, in1, s0, s1, imm2:
            ((in0 * in1 + s0) * in1 + s1).astype(np.float32))
    # P2GATE: out = (v >= imm2) ? (h*v + s0)*v + s1 : 0
    reg("ANT_DI_P2GATE",
        select(Src1 >= C2, (Src0 * Src1 + C0) * Src1 + C1, Zero),
        lambda in0, in1, s0, s1, imm2:
            np.where(in1 >= imm2,
                     (in0 * in1 + s0) * in1 + s1, 0.0).astype(np.float32))
    # P4T: out = ((s0*x + s1)*x + imm2)*x + in1[tensor]  (Horner + tensor tail)
    reg("ANT_DI_P4T",
        ((Src0 * C0 + C1) * Src0 + C2) * Src0 + Src1,
        lambda in0, in1, s0, s1, imm2:
            (((in0 * s0 + s1) * in0 + imm2) * in0 + in1).astype(np.float32))
    return _OPS_CACHE


# ------------------------------------------------------------- kernel build
_BUILD_CACHE = {}


def _build_nc(C):
    import concourse.bacc as bacc
    import concourse.mybir as mybir
    from concourse.tile import TileContext

    ops = _get_ops()
    P3F, P3 = ops["ANT_DI_P3F"], ops["ANT_DI_P3"]
    P2E, P2GATE = ops["ANT_DI_P2E"], ops["ANT_DI_P2GATE"]
    PGATE, DGATE0 = ops["ANT_DI_PGATE"], ops["ANT_DI_DGATE0"]
    P4T = ops["ANT_DI_P4T"]

    f32 = mybir.dt.float32
    AF = mybir.ActivationFunctionType

    nc = bacc.Bacc("TRN2", target_bir_lowering=False)
    x_ext = nc.dram_tensor("x", [N_TILES, P, TILE_N], f32, kind="ExternalInput")
    y_ext = nc.dram_tensor("y", [N_TILES, P, TILE_N], f32, kind="ExternalOutput")

    def cst(v):
        return float(np.float32(v))

    def make_const(v):
        v = cst(v)
        key = (f32, v)
        if key not in nc.const_aps.aps:
            t = nc.alloc_sbuf_tensor(
                f"constr-{len(nc.const_aps.aps)}", [P, 1], f32)
            nc.gpsimd.memset(t.ap(), v)
            nc.const_aps.aps[key] = t.ap()
        return nc.const_aps.tensor(v, (P, 1), f32)

    # Pre-register every constant that must live in SBUF: the C3-spilled
    # 4th Horner coefficients and the activation bias.
    for v in (-6.0, C["cf0"]):
        make_const(v)
    nc.all_engine_barrier()

    with TileContext(nc) as tc:
        with tc.tile_pool(name="io", bufs=3) as iop, \
             tc.tile_pool(name="acts", bufs=2) as actp, \
             tc.tile_pool(name="tmp", bufs=2) as tmpp:
            czero = make_const(0.0)
            for i in range(N_TILES):
                xt = iop.tile([P, TILE_N], f32, tag="x")
                nc.sync.dma_start(out=xt[:], in_=x_ext[i])

                s = actp.tile([P, TILE_N], f32, tag="s")
                u = actp.tile([P, TILE_N], f32, tag="u")
                E = actp.tile([P, TILE_N], f32, tag="E")
                L2 = actp.tile([P, TILE_N], f32, tag="L2")
                w = actp.tile([P, TILE_N], f32, tag="w")
                mf = actp.tile([P, TILE_N], f32, tag="mf")
                yfl = actp.tile([P, TILE_N], f32, tag="yfl")
                nc.scalar.activation(s[:], xt[:], AF.Abs)
                nc.scalar.activation(u[:], xt[:], AF.Square)
                nc.scalar.activation(E[:], u[:], AF.Exp)
                nc.scalar.activation(L2[:], u[:], AF.Ln)
                nc.scalar.activation(w[:], L2[:], AF.Exp, scale=-1.0)
                nc.scalar.activation(mf[:], s[:], AF.Relu, bias=-6.0)
                # yfl = -0.25*L2 + cf0 (the linear part of the far branch)
                nc.scalar.activation(yfl[:], L2[:], AF.Identity,
                                     scale=-0.25, bias=cst(C["cf0"]))

                def c3ap(v):
                    return make_const(v)

                def poly_chain(var, coefs_hi_to_lo):
                    """Horner chain over groups of 3 coefficients (P3F then
                    P3 passes); returns h after consuming all groups."""
                    cs = list(coefs_hi_to_lo)
                    h1 = tmpp.tile([P, TILE_N], f32, tag="h4")
                    nc.vector._custom_dve(
                        P3F, out=h1[:], in0=var[:],
                        s0=cst(cs[0]), s1=cst(cs[1]), imm2=cst(cs[2]))
                    h = h1
                    for g in range(3, len(cs), 3):
                        h2 = tmpp.tile([P, TILE_N], f32, tag="h3" if g == 3 else "h5")
                        nc.vector._custom_dve(
                            P3, out=h2[:], in0=h[:], in1=var[:],
                            s0=cst(cs[g]), s1=cst(cs[g + 1]), imm2=cst(cs[g + 2]))
                        h = h2
                    return h

                # ---- Q: base bin + gated deltas ----
                Q = tmpp.tile([P, TILE_N], f32, tag="Q")
                h = poly_chain(s, C["base"][7:1:-1])
                nc.vector._custom_dve(P2E, out=Q[:], in0=h[:], in1=s[:],
                                      s0=cst(C["base"][1]), s1=cst(C["base"][0]))
                for dd, th in ((C["d2"], C["t1"]),
                               (C["d1"], C["t2"]),
                               (C["d0"], C["t3"])):
                    h = poly_chain(s, dd[7:1:-1])
                    F = tmpp.tile([P, TILE_N], f32, tag="F")
                    nc.vector._custom_dve(P2GATE, out=F[:], in0=h[:], in1=s[:],
                                          s0=cst(dd[1]), s1=cst(dd[0]),
                                          imm2=cst(th))
                    nc.gpsimd.tensor_add(out=Q[:], in0=Q[:], in1=F[:])

                # ---- far branch: yfar = far_poly(w)*w + yfl, merged by mf ----
                cc, cb, ca = C["far_poly"]
                yfar = tmpp.tile([P, TILE_N], f32, tag="yfar")
                nc.vector._custom_dve(P4T, out=yfar[:], in0=w[:], in1=yfl[:],
                                      s0=cst(cc), s1=cst(cb), imm2=cst(ca))
                nc.vector.copy_predicated(
                    Q[:], mf[:].bitcast(mybir.dt.int32), yfar[:])

                # ---- Dawson: 2 gated deg-8 segments ----
                D = tmpp.tile([P, TILE_N], f32, tag="D")
                h = poly_chain(u, C["r1"][::-1])
                nc.vector._custom_dve(DGATE0, out=D[:], in0=h[:], in1=xt[:],
                                      s1=cst(DS1))
                Dx = tmpp.tile([P, TILE_N], f32, tag="Dx")
                h = poly_chain(w, C["r2"][::-1])
                nc.vector._custom_dve(PGATE, out=Dx[:], in0=h[:], in1=xt[:],
                                      s0=0.0, s1=cst(DS1))
                nc.gpsimd.tensor_add(out=D[:], in0=D[:], in1=Dx[:])

                # ---- combine: out = Q + E*D ----
                z = tmpp.tile([P, TILE_N], f32, tag="z")
                nc.gpsimd.tensor_mul(out=z[:], in0=E[:], in1=D[:])
                outt = iop.tile([P, TILE_N], f32, tag="y")
                nc.gpsimd.tensor_add(out=outt[:], in0=Q[:], in1=z[:])
                nc.sync.dma_start(out=y_ext[i], in_=outt[:])
    nc.compile()
    return nc


def _get_nc(cheb_G_neg):
    key = np.asarray(cheb_G_neg, dtype=np.float32).tobytes()
    if key not in _BUILD_CACHE:
        C = build_constants(np.asarray(cheb_G_neg, dtype=np.float32))
        _BUILD_CACHE[key] = _build_nc(C)
    return _BUILD_CACHE[key]


# ------------------------------------------------------------- entry points
def _run(x, cheb_G_neg, **spmd_kwargs):
    from concourse.bass_utils import run_bass_kernel_spmd
    nc = _get_nc(cheb_G_neg)
    x = np.ascontiguousarray(np.asarray(x, dtype=np.float32))
    shards = x.reshape(N_CORES, N_TILES, P, TILE_N)
    in_maps = [{"x": shards[i]} for i in range(N_CORES)]
    res = run_bass_kernel_spmd(nc, in_maps, list(range(N_CORES)), **spmd_kwargs)
    out = np.stack([np.asarray(res.results[i]["y"]) for i in range(N_CORES)])
    return out.reshape(FULL_SHAPE), res


def kernel(x, cheb_G_neg):
    out, _ = _run(x, cheb_G_neg)
    return out


def kernel_timed(x, cheb_G_neg, **kw):
    return _run(x, cheb_G_neg, trace=True, **kw)


# revision 16
# speedup vs baseline: 1.3911x; 1.0076x over previous
"""Trainium2 Bass kernel for nn_DawsonIntegrate.

Computes, elementwise over x (f32):
    s = |x|;  near (s<=6): piecewise-Chebyshev table poly;  far (s>6): asymptotic
    plus, for x>0: (pi/2)*erfi(x) = sqrt(pi)*exp(x^2)*dawson(x).

Strategy (per core, data-parallel over 8 cores on the leading batch dim):
  ACT:   s=|x|, u=x^2, E=exp(u), L2=ln(u), w=exp(-L2)=1/x^2, mf=relu(s-6)
  DVE:   custom fused-Horner ops evaluate the 4 bin polynomials (exact
         monomial conversion of the Chebyshev table, delta-telescoped with
         in-op gating), the asymptotic far branch, and a 4-segment
         minimax fit of sqrt(pi)*dawson(x)/x (in u=x^2 or w=1/x^2).
  GPSIMD: tensor adds / final combine.
All table-dependent coefficients are computed on the host from cheb_G_neg
and baked into the program as immediates.
"""
import numpy as np

# ---------------------------------------------------------------- constants
DIV, DEG = 4, 8
CHEB_XMIN = -6.0
DELTA = 1.5
EULER_GAMMA = 0.5772156649015329
SQRT_PI = float(np.sqrt(np.pi))
DS1 = 2.25                         # dawson segment boundary (f32-exact)
XMAX = 9.7

FULL_SHAPE = (16, 2048, 1024)
N_CORES = 8
P = 128
TILE_N = 1024
ROWS_PER_CORE = FULL_SHAPE[0] // N_CORES  # 2
ELEMS_PER_CORE = ROWS_PER_CORE * FULL_SHAPE[1] * FULL_SHAPE[2]
N_TILES = ELEMS_PER_CORE // (P * TILE_N)  # 32


# ------------------------------------------------------- host-side numerics
def _dawsn_f64(x):
    """Dawson function via the stable all-positive erfi Maclaurin series,
    float64: D(x) = exp(-x^2) * sqrt(pi)/2 * erfi(x)."""
    x = np.asarray(x, dtype=np.float64)
    x2 = x * x
    t = x.copy()
    ssum = x.copy()
    for n in range(1, 400):
        t = t * x2 / n
        ssum = ssum + t / (2.0 * n + 1.0)
    # D = exp(-x^2) * series  (series = sqrt(pi)/2*erfi / ... cancels)
    return np.exp(-x2) * ssum


def _ref_bin_f32(s):
    v = (-s).astype(np.float32)
    t = ((v - np.float32(CHEB_XMIN)) / np.float32(DELTA)).astype(np.float32)
    b = np.ceil(t).astype(np.int32) - 1
    return np.clip(b, 0, DIV - 1)


def _effective_thresholds():
    ts = []
    for nominal in (1.5, 3.0, 4.5):
        lo = np.float32(nominal)
        cands = [lo]
        c = lo
        for _ in range(8):
            c = np.nextafter(c, np.float32(0), dtype=np.float32)
            cands.append(c)
        c = lo
        for _ in range(8):
            c = np.nextafter(c, np.float32(100), dtype=np.float32)
            cands.append(c)
        cands = np.sort(np.array(cands, dtype=np.float32))
        b = _ref_bin_f32(cands)
        assert b[0] == b[-1] + 1
        ts.append(float(cands[np.argmax(b == b[-1])]))
    return ts


def _minimax_fit(var, t, deg, iters=15):
    V = np.vander(var, deg + 1, increasing=True)
    w = 1.0 / np.abs(t)
    best = None
    for _ in range(iters):
        c, *_ = np.linalg.lstsq(V * w[:, None], t * w, rcond=None)
        r = np.abs((V @ c - t) / t)
        if best is None or r.max() < best[1]:
            best = (c, r.max())
        w = w * (1 + r / (r.max() + 1e-30))
    return best[0]


def _dawson_fit(xlo, xhi, basis, deg=6, n=6001):
    x = np.linspace(max(xlo, 1e-9), xhi, n)
    t = SQRT_PI * _dawsn_f64(x) / x
    var = x * x if basis == "u" else 1.0 / (x * x)
    return _minimax_fit(var, t, deg)


def build_constants(cheb_G_neg):
    t1, t2, t3 = _effective_thresholds()
    A = np.zeros((DIV, DEG))
    for b in range(DIV):
        mono_v = np.polynomial.chebyshev.cheb2poly(
            np.asarray(cheb_G_neg[b], dtype=np.float64))
        mono_v = np.concatenate([mono_v, np.zeros(DEG - len(mono_v))])
        A[b] = mono_v * ((-1.0) ** np.arange(DEG))
    return dict(
        t1=t1, t2=t2, t3=t3,
        base=A[3], d2=A[2] - A[3], d1=A[1] - A[2], d0=A[0] - A[1],
        r1=_dawson_fit(0, DS1, "u", deg=8),
        r2=_dawson_fit(DS1, XMAX, "w", deg=8),
        cf0=-0.25 * EULER_GAMMA - 0.5 * np.log(2.0),
        far_poly=(-5.0 / 32.0, 3.0 / 32.0, -1.0 / 8.0),  # cc, cb, ca (in w)
    )


# ------------------------------------------------- custom DVE op registration
_OPS_CACHE = {}


def _get_ops():
    if _OPS_CACHE:
        return _OPS_CACHE
    from concourse.dve_spec import (
        Spec, Src0, Src1, C0, C1, C2, C3, Zero, lower, select,
        _spill_c3_to_src1, _has_src1,
    )
    import concourse.dve_ops as dve_ops_mod
    from concourse.dve_ops import DveOp, OPS
    from concourse.dve_uop import DveOpSpec

    existing = {op.name for op in OPS}

    def reg(name, body, reference):
        spec = Spec(body=body, reference=reference)
        shas = {}
        for ver in ("v3", "v4"):
            shas[ver] = DveOpSpec(
                name=name, opcode=0, uops=lower(spec, ver=ver),
                rd1_en=_has_src1(spec),
            ).sha(ver)
        op = DveOp(name, spec, False, shas)
        if name not in existing:
            OPS.append(op)
        else:  # replace (idempotent re-import)
            for i, o in enumerate(OPS):
                if o.name == name:
                    OPS[i] = op
        # refresh the import-time snapshots keyed off OPS
        dve_ops_mod.CUSTOM_DVE_SPECS[name] = spec
        dve_ops_mod._SUB_OPCODE_FOR_NAME.clear()
        dve_ops_mod._SUB_OPCODE_FOR_NAME.update({
            o.name: dve_ops_mod._CUSTOM_DVE_ROW_BASE + i
            for i, o in enumerate(OPS)
        })
        assert max(dve_ops_mod._SUB_OPCODE_FOR_NAME.values()) < 0x20
        _OPS_CACHE[name] = op
        return op

    def _b(a):  # broadcast [P,1] -> [P,N]
        a = np.asarray(a)
        return a if a.ndim == 0 else a.reshape(a.shape[0], -1)[:, :1]

    # P4: h = ((s0*x + s1)*x + imm2)*x + in1   (4-coef Horner, single stream)
    reg("ANT_DI_P4",
        _spill_c3_to_src1(((Src0 * C0 + C1) * Src0 + C2) * Src0 + C3),
        lambda in0, in1, s0, s1, imm2:
            (((in0 * s0 + s1) * in0 + imm2) * in0 + _b(in1)).astype(np.float32))
    # P3: h = ((h*v + s0)*v + s1)*v + imm2    (3-coef Horner continue)
    reg("ANT_DI_P3",
        ((Src0 * Src1 + C0) * Src1 + C1) * Src1 + C2,
        lambda in0, in1, s0, s1, imm2:
            (((in0 * in1 + s0) * in1 + s1) * in1 + imm2).astype(np.float32))
    # PEND: out = h*v + s0
    reg("ANT_DI_PEND",
        Src0 * Src1 + C0,
        lambda in0, in1, s0, s1, imm2: (in0 * in1 + s0).astype(np.float32))
    # PGATE: out = (v >= s1) ? h*v + s0 : 0
    reg("ANT_DI_PGATE",
        select(Src1 >= C1, Src0 * Src1 + C0, Zero),
        lambda in0, in1, s0, s1, imm2:
            np.where(in1 >= s1, in0 * in1 + s0, 0.0).astype(np.float32))
    # DGATE0: out = (0 < v < s1) ? h*v : 0
    reg("ANT_DI_DGATE0",
        select((Src1 > Zero) & (Src1 < C1), Src0 * Src1, Zero),
        lambda in0, in1, s0, s1, imm2:
            np.where((in1 > 0) & (in1 < s1), in0 * in1, 0.0).astype(np.float32))
    # DGATE2: out = (s0 <= v < s1) ? h*v : 0
    reg("ANT_DI_DGATE2",
        select((Src1 >= C0) & (Src1 < C1), Src0 * Src1, Zero),
        lambda in0, in1, s0, s1, imm2:
            np.where((in1 >= s0) & (in1 < s1), in0 * in1, 0.0).astype(np.float32))
    # P3F: h = (s0*x + s1)*x + imm2   (fresh 3-coef Horner, single stream)
    reg("ANT_DI_P3F",
        (Src0 * C0 + C1) * Src0 + C2,
        lambda in0, in1, s0, s1, imm2:
            ((in0 * s0 + s1) * in0 + imm2).astype(np.float32))
    # P2E: out = (h*v + s0)*v + s1
    reg("ANT_DI_P2E",
        (Src0 * Src1 + C0) * Src1 + C1,
        lambda in0, in1, s0, s1, imm2:
            ((in0 * in1 + s0) * in1 + s1).astype(np.float32))
    # P2GATE: out = (v >= imm2) ? (h*v + s0)*v + s1 : 0
    reg("ANT_DI_P2GATE",
        select(Src1 >= C2, (Src0 * Src1 + C0) * Src1 + C1, Zero),
        lambda in0, in1, s0, s1, imm2:
            np.where(in1 >= imm2,
                     (in0 * in1 + s0) * in1 + s1, 0.0).astype(np.float32))
    # P4T: out = ((s0*x + s1)*x + imm2)*x + in1[tensor]  (Horner + tensor tail)
    reg("ANT_DI_P4T",
        ((Src0 * C0 + C1) * Src0 + C2) * Src0 + Src1,
        lambda in0, in1, s0, s1, imm2:
            (((in0 * s0 + s1) * in0 + imm2) * in0 + in1).astype(np.float32))
    return _OPS_CACHE


# ------------------------------------------------------------- kernel build
_BUILD_CACHE = {}


def _build_nc(C):
    import concourse.bacc as bacc
    import concourse.mybir as mybir
    from concourse.tile import TileContext

    ops = _get_ops()
    P3F, P3 = ops["ANT_DI_P3F"], ops["ANT_DI_P3"]
    P2E, P2GATE = ops["ANT_DI_P2E"], ops["ANT_DI_P2GATE"]
    PGATE, DGATE0 = ops["ANT_DI_PGATE"], ops["ANT_DI_DGATE0"]
    P4T = ops["ANT_DI_P4T"]

    f32 = mybir.dt.float32
    AF = mybir.ActivationFunctionType

    nc = bacc.Bacc("TRN2", target_bir_lowering=False)
    x_ext = nc.dram_tensor("x", [N_TILES, P, TILE_N], f32, kind="ExternalInput")
    y_ext = nc.dram_tensor("y", [N_TILES, P, TILE_N], f32, kind="ExternalOutput")

    def cst(v):
        return float(np.float32(v))

    def make_const(v):
        v = cst(v)
        key = (f32, v)
        if key not in nc.const_aps.aps:
            t = nc.alloc_sbuf_tensor(
                f"constr-{len(nc.const_aps.aps)}", [P, 1], f32)
            nc.gpsimd.memset(t.ap(), v)
            nc.const_aps.aps[key] = t.ap()
        return nc.const_aps.tensor(v, (P, 1), f32)

    # Pre-register every constant that must live in SBUF: the C3-spilled
    # 4th Horner coefficients and the activation bias.
    for v in (-6.0, C["cf0"]):
        make_const(v)
    nc.all_engine_barrier()

    with TileContext(nc) as tc:
        with tc.tile_pool(name="io", bufs=3) as iop, \
             tc.tile_pool(name="acts", bufs=2) as actp, \
             tc.tile_pool(name="tmp", bufs=2) as tmpp:
            czero = make_const(0.0)
            for i in range(N_TILES):
                xt = iop.tile([P, TILE_N], f32, tag="x")
                nc.sync.dma_start(out=xt[:], in_=x_ext[i])

                s = actp.tile([P, TILE_N], f32, tag="s")
                u = actp.tile([P, TILE_N], f32, tag="u")
                E = actp.tile([P, TILE_N], f32, tag="E")
                L2 = actp.tile([P, TILE_N], f32, tag="L2")
                w = actp.tile([P, TILE_N], f32, tag="w")
                mf = actp.tile([P, TILE_N], f32, tag="mf")
                yfl = actp.tile([P, TILE_N], f32, tag="yfl")
                nc.scalar.activation(s[:], xt[:], AF.Abs)
                nc.scalar.activation(u[:], xt[:], AF.Square)
                nc.scalar.activation(E[:], u[:], AF.Exp)
                nc.scalar.activation(L2[:], u[:], AF.Ln)
                nc.scalar.activation(w[:], L2[:], AF.Exp, scale=-1.0)
                nc.scalar.activation(mf[:], s[:], AF.Relu, bias=-6.0)
                # yfl = -0.25*L2 + cf0 (the linear part of the far branch)
                nc.scalar.activation(yfl[:], L2[:], AF.Identity,
                                     scale=-0.25, bias=cst(C["cf0"]))

                def c3ap(v):
                    return make_const(v)

                def poly_chain(var, coefs_hi_to_lo):
                    """Horner chain over groups of 3 coefficients (P3F then
                    P3 passes); returns h after consuming all groups."""
                    cs = list(coefs_hi_to_lo)
                    h1 = tmpp.tile([P, TILE_N], f32, tag="h4")
                    nc.vector._custom_dve(
                        P3F, out=h1[:], in0=var[:],
                        s0=cst(cs[0]), s1=cst(cs[1]), imm2=cst(cs[2]))
                    h = h1
                    for g in range(3, len(cs), 3):
                        h2 = tmpp.tile([P, TILE_N], f32, tag="h3" if g == 3 else "h5")
                        nc.vector._custom_dve(
                            P3, out=h2[:], in0=h[:], in1=var[:],
                            s0=cst(cs[g]), s1=cst(cs[g + 1]), imm2=cst(cs[g + 2]))
                        h = h2
                    return h

                # ---- far branch (emitted early; merged after Q-adds) ----
                cc, cb, ca = C["far_poly"]
                yfar = tmpp.tile([P, TILE_N], f32, tag="yfar")
                nc.vector._custom_dve(P4T, out=yfar[:], in0=w[:], in1=yfl[:],
                                      s0=cst(cc), s1=cst(cb), imm2=cst(ca))

                # ---- Q: base bin + gated deltas ----
                Q = tmpp.tile([P, TILE_N], f32, tag="Q")
                h = poly_chain(s, [0.0] + list(C["base"][7:2:-1]))
                nc.vector._custom_dve(P3, out=Q[:], in0=h[:], in1=s[:],
                                      s0=cst(C["base"][2]), s1=cst(C["base"][1]),
                                      imm2=cst(C["base"][0]))
                for dd, th in ((C["d2"], C["t1"]),
                               (C["d1"], C["t2"]),
                               (C["d0"], C["t3"])):
                    h = poly_chain(s, dd[7:1:-1])
                    F = tmpp.tile([P, TILE_N], f32, tag="F")
                    nc.vector._custom_dve(P2GATE, out=F[:], in0=h[:], in1=s[:],
                                          s0=cst(dd[1]), s1=cst(dd[0]),
                                          imm2=cst(th))
                    nc.gpsimd.tensor_add(out=Q[:], in0=Q[:], in1=F[:])

                nc.vector.copy_predicated(
                    Q[:], mf[:].bitcast(mybir.dt.int32), yfar[:])

                # ---- Dawson: 2 gated deg-8 segments ----
                D = tmpp.tile([P, TILE_N], f32, tag="D")
                h = poly_chain(u, C["r1"][::-1])
                nc.vector._custom_dve(DGATE0, out=D[:], in0=h[:], in1=xt[:],
                                      s1=cst(DS1))
                Dx = tmpp.tile([P, TILE_N], f32, tag="Dx")
                h = poly_chain(w, C["r2"][::-1])
                nc.vector._custom_dve(PGATE, out=Dx[:], in0=h[:], in1=xt[:],
                                      s0=0.0, s1=cst(DS1))
                nc.gpsimd.tensor_add(out=D[:], in0=D[:], in1=Dx[:])

                # ---- combine: out = Q + E*D ----
                z = tmpp.tile([P, TILE_N], f32, tag="z")
                nc.gpsimd.tensor_mul(out=z[:], in0=E[:], in1=D[:])
                outt = iop.tile([P, TILE_N], f32, tag="y")
                nc.gpsimd.tensor_add(out=outt[:], in0=Q[:], in1=z[:])
                nc.sync.dma_start(out=y_ext[i], in_=outt[:])
    nc.compile()
    return nc


def _get_nc(cheb_G_neg):
    key = np.asarray(cheb_G_neg, dtype=np.float32).tobytes()
    if key not in _BUILD_CACHE:
        C = build_constants(np.asarray(cheb_G_neg, dtype=np.float32))
        _BUILD_CACHE[key] = _build_nc(C)
    return _BUILD_CACHE[key]


# ------------------------------------------------------------- entry points
def _run(x, cheb_G_neg, **spmd_kwargs):
    from concourse.bass_utils import run_bass_kernel_spmd
    nc = _get_nc(cheb_G_neg)
    x = np.ascontiguousarray(np.asarray(x, dtype=np.float32))
    shards = x.reshape(N_CORES, N_TILES, P, TILE_N)
    in_maps = [{"x": shards[i]} for i in range(N_CORES)]
    res = run_bass_kernel_spmd(nc, in_maps, list(range(N_CORES)), **spmd_kwargs)
    out = np.stack([np.asarray(res.results[i]["y"]) for i in range(N_CORES)])
    return out.reshape(FULL_SHAPE), res


def kernel(x, cheb_G_neg):
    out, _ = _run(x, cheb_G_neg)
    return out


def kernel_timed(x, cheb_G_neg, **kw):
    return _run(x, cheb_G_neg, trace=True, **kw)


# revision 17
# speedup vs baseline: 1.4491x; 1.0417x over previous
"""Trainium2 Bass kernel for nn_DawsonIntegrate.

Computes, elementwise over x (f32):
    s = |x|;  near (s<=6): piecewise-Chebyshev table poly;  far (s>6): asymptotic
    plus, for x>0: (pi/2)*erfi(x) = sqrt(pi)*exp(x^2)*dawson(x).

Strategy (per core, data-parallel over 8 cores on the leading batch dim):
  ACT:   s=|x|, u=x^2, E=exp(u), L2=ln(u), w=exp(-L2)=1/x^2, mf=relu(s-6)
  DVE:   custom fused-Horner ops evaluate the 4 bin polynomials (exact
         monomial conversion of the Chebyshev table, delta-telescoped with
         in-op gating), the asymptotic far branch, and a 4-segment
         minimax fit of sqrt(pi)*dawson(x)/x (in u=x^2 or w=1/x^2).
  GPSIMD: tensor adds / final combine.
All table-dependent coefficients are computed on the host from cheb_G_neg
and baked into the program as immediates.
"""
import numpy as np

# ---------------------------------------------------------------- constants
DIV, DEG = 4, 8
CHEB_XMIN = -6.0
DELTA = 1.5
EULER_GAMMA = 0.5772156649015329
SQRT_PI = float(np.sqrt(np.pi))
DS1 = 2.25                         # dawson segment boundary (f32-exact)
XMAX = 9.7

FULL_SHAPE = (16, 2048, 1024)
N_CORES = 8
P = 128
TILE_N = 1024
ROWS_PER_CORE = FULL_SHAPE[0] // N_CORES  # 2
ELEMS_PER_CORE = ROWS_PER_CORE * FULL_SHAPE[1] * FULL_SHAPE[2]
N_TILES = ELEMS_PER_CORE // (P * TILE_N)  # 32


# ------------------------------------------------------- host-side numerics
def _dawsn_f64(x):
    """Dawson function via the stable all-positive erfi Maclaurin series,
    float64: D(x) = exp(-x^2) * sqrt(pi)/2 * erfi(x)."""
    x = np.asarray(x, dtype=np.float64)
    x2 = x * x
    t = x.copy()
    ssum = x.copy()
    for n in range(1, 400):
        t = t * x2 / n
        ssum = ssum + t / (2.0 * n + 1.0)
    # D = exp(-x^2) * series  (series = sqrt(pi)/2*erfi / ... cancels)
    return np.exp(-x2) * ssum


def _ref_bin_f32(s):
    v = (-s).astype(np.float32)
    t = ((v - np.float32(CHEB_XMIN)) / np.float32(DELTA)).astype(np.float32)
    b = np.ceil(t).astype(np.int32) - 1
    return np.clip(b, 0, DIV - 1)


def _effective_thresholds():
    ts = []
    for nominal in (1.5, 3.0, 4.5):
        lo = np.float32(nominal)
        cands = [lo]
        c = lo
        for _ in range(8):
            c = np.nextafter(c, np.float32(0), dtype=np.float32)
            cands.append(c)
        c = lo
        for _ in range(8):
            c = np.nextafter(c, np.float32(100), dtype=np.float32)
            cands.append(c)
        cands = np.sort(np.array(cands, dtype=np.float32))
        b = _ref_bin_f32(cands)
        assert b[0] == b[-1] + 1
        ts.append(float(cands[np.argmax(b == b[-1])]))
    return ts


def _minimax_fit(var, t, deg, iters=15):
    V = np.vander(var, deg + 1, increasing=True)
    w = 1.0 / np.abs(t)
    best = None
    for _ in range(iters):
        c, *_ = np.linalg.lstsq(V * w[:, None], t * w, rcond=None)
        r = np.abs((V @ c - t) / t)
        if best is None or r.max() < best[1]:
            best = (c, r.max())
        w = w * (1 + r / (r.max() + 1e-30))
    return best[0]


def _dawson_fit(xlo, xhi, basis, deg=6, n=6001):
    x = np.linspace(max(xlo, 1e-9), xhi, n)
    t = SQRT_PI * _dawsn_f64(x) / x
    var = x * x if basis == "u" else 1.0 / (x * x)
    return _minimax_fit(var, t, deg)


def build_constants(cheb_G_neg):
    t1, t2, t3 = _effective_thresholds()
    A = np.zeros((DIV, DEG))
    for b in range(DIV):
        mono_v = np.polynomial.chebyshev.cheb2poly(
            np.asarray(cheb_G_neg[b], dtype=np.float64))
        mono_v = np.concatenate([mono_v, np.zeros(DEG - len(mono_v))])
        A[b] = mono_v * ((-1.0) ** np.arange(DEG))
    return dict(
        t1=t1, t2=t2, t3=t3,
        base=A[3], d2=A[2] - A[3], d1=A[1] - A[2], d0=A[0] - A[1],
        r1=_dawson_fit(0, DS1, "u", deg=8),
        r2=_dawson_fit(DS1, XMAX, "w", deg=8),
        cf0=-0.25 * EULER_GAMMA - 0.5 * np.log(2.0),
        far_poly=(-5.0 / 32.0, 3.0 / 32.0, -1.0 / 8.0),  # cc, cb, ca (in w)
    )


# ------------------------------------------------- custom DVE op registration
_OPS_CACHE = {}


def _get_ops():
    if _OPS_CACHE:
        return _OPS_CACHE
    from concourse.dve_spec import (
        Spec, Src0, Src1, C0, C1, C2, C3, Zero, lower, select,
        _spill_c3_to_src1, _has_src1,
    )
    import concourse.dve_ops as dve_ops_mod
    from concourse.dve_ops import DveOp, OPS
    from concourse.dve_uop import DveOpSpec

    existing = {op.name for op in OPS}

    def reg(name, body, reference):
        spec = Spec(body=body, reference=reference)
        shas = {}
        for ver in ("v3", "v4"):
            shas[ver] = DveOpSpec(
                name=name, opcode=0, uops=lower(spec, ver=ver),
                rd1_en=_has_src1(spec),
            ).sha(ver)
        op = DveOp(name, spec, False, shas)
        if name not in existing:
            OPS.append(op)
        else:  # replace (idempotent re-import)
            for i, o in enumerate(OPS):
                if o.name == name:
                    OPS[i] = op
        # refresh the import-time snapshots keyed off OPS
        dve_ops_mod.CUSTOM_DVE_SPECS[name] = spec
        dve_ops_mod._SUB_OPCODE_FOR_NAME.clear()
        dve_ops_mod._SUB_OPCODE_FOR_NAME.update({
            o.name: dve_ops_mod._CUSTOM_DVE_ROW_BASE + i
            for i, o in enumerate(OPS)
        })
        assert max(dve_ops_mod._SUB_OPCODE_FOR_NAME.values()) < 0x20
        _OPS_CACHE[name] = op
        return op

    def _b(a):  # broadcast [P,1] -> [P,N]
        a = np.asarray(a)
        return a if a.ndim == 0 else a.reshape(a.shape[0], -1)[:, :1]

    # P4: h = ((s0*x + s1)*x + imm2)*x + in1   (4-coef Horner, single stream)
    reg("ANT_DI_P4",
        _spill_c3_to_src1(((Src0 * C0 + C1) * Src0 + C2) * Src0 + C3),
        lambda in0, in1, s0, s1, imm2:
            (((in0 * s0 + s1) * in0 + imm2) * in0 + _b(in1)).astype(np.float32))
    # P3: h = ((h*v + s0)*v + s1)*v + imm2    (3-coef Horner continue)
    reg("ANT_DI_P3",
        ((Src0 * Src1 + C0) * Src1 + C1) * Src1 + C2,
        lambda in0, in1, s0, s1, imm2:
            (((in0 * in1 + s0) * in1 + s1) * in1 + imm2).astype(np.float32))
    # PEND: out = h*v + s0
    reg("ANT_DI_PEND",
        Src0 * Src1 + C0,
        lambda in0, in1, s0, s1, imm2: (in0 * in1 + s0).astype(np.float32))
    # PGATE: out = (v >= s1) ? h*v + s0 : 0
    reg("ANT_DI_PGATE",
        select(Src1 >= C1, Src0 * Src1 + C0, Zero),
        lambda in0, in1, s0, s1, imm2:
            np.where(in1 >= s1, in0 * in1 + s0, 0.0).astype(np.float32))
    # DGATE0: out = (0 < v < s1) ? h*v : 0
    reg("ANT_DI_DGATE0",
        select((Src1 > Zero) & (Src1 < C1), Src0 * Src1, Zero),
        lambda in0, in1, s0, s1, imm2:
            np.where((in1 > 0) & (in1 < s1), in0 * in1, 0.0).astype(np.float32))
    # DGATE2: out = (s0 <= v < s1) ? h*v : 0
    reg("ANT_DI_DGATE2",
        select((Src1 >= C0) & (Src1 < C1), Src0 * Src1, Zero),
        lambda in0, in1, s0, s1, imm2:
            np.where((in1 >= s0) & (in1 < s1), in0 * in1, 0.0).astype(np.float32))
    # P3F: h = (s0*x + s1)*x + imm2   (fresh 3-coef Horner, single stream)
    reg("ANT_DI_P3F",
        (Src0 * C0 + C1) * Src0 + C2,
        lambda in0, in1, s0, s1, imm2:
            ((in0 * s0 + s1) * in0 + imm2).astype(np.float32))
    # P2E: out = (h*v + s0)*v + s1
    reg("ANT_DI_P2E",
        (Src0 * Src1 + C0) * Src1 + C1,
        lambda in0, in1, s0, s1, imm2:
            ((in0 * in1 + s0) * in1 + s1).astype(np.float32))
    # P2GATE: out = (v >= imm2) ? (h*v + s0)*v + s1 : 0
    reg("ANT_DI_P2GATE",
        select(Src1 >= C2, (Src0 * Src1 + C0) * Src1 + C1, Zero),
        lambda in0, in1, s0, s1, imm2:
            np.where(in1 >= imm2,
                     (in0 * in1 + s0) * in1 + s1, 0.0).astype(np.float32))
    # P4T: out = ((s0*x + s1)*x + imm2)*x + in1[tensor]  (Horner + tensor tail)
    reg("ANT_DI_P4T",
        ((Src0 * C0 + C1) * Src0 + C2) * Src0 + Src1,
        lambda in0, in1, s0, s1, imm2:
            (((in0 * s0 + s1) * in0 + imm2) * in0 + in1).astype(np.float32))
    return _OPS_CACHE


# ------------------------------------------------------------- kernel build
_BUILD_CACHE = {}


def _build_nc(C):
    import concourse.bacc as bacc
    import concourse.mybir as mybir
    from concourse.tile import TileContext

    ops = _get_ops()
    P3F, P3 = ops["ANT_DI_P3F"], ops["ANT_DI_P3"]
    P2E, P2GATE = ops["ANT_DI_P2E"], ops["ANT_DI_P2GATE"]
    PGATE, DGATE0 = ops["ANT_DI_PGATE"], ops["ANT_DI_DGATE0"]
    P4T = ops["ANT_DI_P4T"]

    f32 = mybir.dt.float32
    AF = mybir.ActivationFunctionType

    nc = bacc.Bacc("TRN2", target_bir_lowering=False)
    x_ext = nc.dram_tensor("x", [N_TILES, P, TILE_N], f32, kind="ExternalInput")
    y_ext = nc.dram_tensor("y", [N_TILES, P, TILE_N], f32, kind="ExternalOutput")

    def cst(v):
        return float(np.float32(v))

    def make_const(v):
        v = cst(v)
        key = (f32, v)
        if key not in nc.const_aps.aps:
            t = nc.alloc_sbuf_tensor(
                f"constr-{len(nc.const_aps.aps)}", [P, 1], f32)
            nc.gpsimd.memset(t.ap(), v)
            nc.const_aps.aps[key] = t.ap()
        return nc.const_aps.tensor(v, (P, 1), f32)

    # Pre-register every constant that must live in SBUF: the C3-spilled
    # 4th Horner coefficients and the activation bias.
    for v in (-6.0, C["cf0"]):
        make_const(v)
    nc.all_engine_barrier()

    with TileContext(nc) as tc:
        with tc.tile_pool(name="io", bufs=3) as iop, \
             tc.tile_pool(name="acts", bufs=2) as actp, \
             tc.tile_pool(name="tmp", bufs=2) as tmpp:
            czero = make_const(0.0)
            for i in range(N_TILES):
                xt = iop.tile([P, TILE_N], f32, tag="x")
                nc.sync.dma_start(out=xt[:], in_=x_ext[i])

                s = actp.tile([P, TILE_N], f32, tag="s")
                u = actp.tile([P, TILE_N], f32, tag="u")
                E = actp.tile([P, TILE_N], f32, tag="E")
                L2 = actp.tile([P, TILE_N], f32, tag="L2")
                w = actp.tile([P, TILE_N], f32, tag="w")
                mf = actp.tile([P, TILE_N], f32, tag="mf")
                yfl = actp.tile([P, TILE_N], f32, tag="yfl")
                nc.scalar.activation(s[:], xt[:], AF.Abs)
                nc.scalar.activation(u[:], xt[:], AF.Square)
                nc.scalar.activation(E[:], u[:], AF.Exp)
                nc.scalar.activation(L2[:], u[:], AF.Ln)
                nc.scalar.activation(w[:], L2[:], AF.Exp, scale=-1.0)
                nc.scalar.activation(mf[:], s[:], AF.Relu, bias=-6.0)
                # yfl = -0.25*L2 + cf0 (the linear part of the far branch)
                nc.scalar.activation(yfl[:], L2[:], AF.Identity,
                                     scale=-0.25, bias=cst(C["cf0"]))

                def c3ap(v):
                    return make_const(v)

                def poly_chain(var, coefs_hi_to_lo):
                    """Horner chain over groups of 3 coefficients (P3F then
                    P3 passes); returns h after consuming all groups."""
                    cs = list(coefs_hi_to_lo)
                    h1 = tmpp.tile([P, TILE_N], f32, tag="h4")
                    nc.vector._custom_dve(
                        P3F, out=h1[:], in0=var[:],
                        s0=cst(cs[0]), s1=cst(cs[1]), imm2=cst(cs[2]))
                    h = h1
                    for g in range(3, len(cs), 3):
                        h2 = tmpp.tile([P, TILE_N], f32, tag="h3" if g == 3 else "h5")
                        nc.vector._custom_dve(
                            P3, out=h2[:], in0=h[:], in1=var[:],
                            s0=cst(cs[g]), s1=cst(cs[g + 1]), imm2=cst(cs[g + 2]))
                        h = h2
                    return h

                # ---- far branch (emitted early; merged after Q-adds) ----
                cc, cb, ca = C["far_poly"]
                yfar = tmpp.tile([P, TILE_N], f32, tag="yfar")
                nc.vector._custom_dve(P4T, out=yfar[:], in0=w[:], in1=yfl[:],
                                      s0=cst(cc), s1=cst(cb), imm2=cst(ca))

                # ---- Q: base bin + gated deltas ----
                Q = tmpp.tile([P, TILE_N], f32, tag="Q")
                h = poly_chain(s, [0.0] + list(C["base"][7:2:-1]))
                nc.vector._custom_dve(P3, out=Q[:], in0=h[:], in1=s[:],
                                      s0=cst(C["base"][2]), s1=cst(C["base"][1]),
                                      imm2=cst(C["base"][0]))
                Fs = []
                for dd, th in ((C["d2"], C["t1"]),
                               (C["d1"], C["t2"]),
                               (C["d0"], C["t3"])):
                    h = poly_chain(s, dd[7:1:-1])
                    F = tmpp.tile([P, TILE_N], f32, tag="F", bufs=4)
                    nc.vector._custom_dve(P2GATE, out=F[:], in0=h[:], in1=s[:],
                                          s0=cst(dd[1]), s1=cst(dd[0]),
                                          imm2=cst(th))
                    Fs.append(F)
                QB = tmpp.tile([P, TILE_N], f32, tag="QB")
                nc.gpsimd.tensor_add(out=QB[:], in0=Fs[1][:], in1=Fs[2][:])
                nc.gpsimd.tensor_add(out=Q[:], in0=Q[:], in1=Fs[0][:])
                nc.gpsimd.tensor_add(out=Q[:], in0=Q[:], in1=QB[:])

                nc.vector.copy_predicated(
                    Q[:], mf[:].bitcast(mybir.dt.int32), yfar[:])

                # ---- Dawson: 2 gated deg-8 segments ----
                D = tmpp.tile([P, TILE_N], f32, tag="D")
                h = poly_chain(u, C["r1"][::-1])
                nc.vector._custom_dve(DGATE0, out=D[:], in0=h[:], in1=xt[:],
                                      s1=cst(DS1))
                Dx = tmpp.tile([P, TILE_N], f32, tag="Dx")
                h = poly_chain(w, C["r2"][::-1])
                nc.vector._custom_dve(PGATE, out=Dx[:], in0=h[:], in1=xt[:],
                                      s0=0.0, s1=cst(DS1))
                nc.gpsimd.tensor_add(out=D[:], in0=D[:], in1=Dx[:])

                # ---- combine: out = Q + E*D ----
                z = tmpp.tile([P, TILE_N], f32, tag="z")
                nc.gpsimd.tensor_mul(out=z[:], in0=E[:], in1=D[:])
                outt = iop.tile([P, TILE_N], f32, tag="y")
                nc.gpsimd.tensor_add(out=outt[:], in0=Q[:], in1=z[:])
                nc.sync.dma_start(out=y_ext[i], in_=outt[:])
    nc.compile()
    return nc


def _get_nc(cheb_G_neg):
    key = np.asarray(cheb_G_neg, dtype=np.float32).tobytes()
    if key not in _BUILD_CACHE:
        C = build_constants(np.asarray(cheb_G_neg, dtype=np.float32))
        _BUILD_CACHE[key] = _build_nc(C)
    return _BUILD_CACHE[key]


# ------------------------------------------------------------- entry points
def _run(x, cheb_G_neg, **spmd_kwargs):
    from concourse.bass_utils import run_bass_kernel_spmd
    nc = _get_nc(cheb_G_neg)
    x = np.ascontiguousarray(np.asarray(x, dtype=np.float32))
    shards = x.reshape(N_CORES, N_TILES, P, TILE_N)
    in_maps = [{"x": shards[i]} for i in range(N_CORES)]
    res = run_bass_kernel_spmd(nc, in_maps, list(range(N_CORES)), **spmd_kwargs)
    out = np.stack([np.asarray(res.results[i]["y"]) for i in range(N_CORES)])
    return out.reshape(FULL_SHAPE), res


def kernel(x, cheb_G_neg):
    out, _ = _run(x, cheb_G_neg)
    return out


def kernel_timed(x, cheb_G_neg, **kw):
    return _run(x, cheb_G_neg, trace=True, **kw)
